# revision 14
# baseline (speedup 1.0000x reference)
"""AttentionWithPairBias distributed Trainium2 kernel (8 NeuronCores).

Sequence-parallel: core c owns query rows i in [128c, 128(c+1)).
Per core: z shard [128, 1024, 128] f32 (64MB -> memory roofline), s and
weights replicated. No collectives.

v2 changes over the 507us baseline (trace-driven):
- Bias matmuls 2-way column-tiled (i split 64+64, tile_position (0,0)
  and (0,64)): LDWEIGHTS of one col-group overlaps the other group's
  in-flight matmul, breaking the LDW->MM drain serialization seen in
  the trace (104+172ns per j).
- z^2 stats: DVE bf16 tensor_tensor halving-tree (2x mode) replaces
  the 1x tensor_reduce (146us). Squares split ScalarE/GpSimd.
- cast prefetch DGE issued after the squares in program order so the
  GpSimd FIFO doesn't stall squares behind the descriptor-gen that
  waits on the transpose (trace showed strict cast/transpose
  alternation from this chain).
- Preamble DMAs moved to the scalar HWDGE ring; sync ring carries only
  the 8 z transposes. Preamble weights/s in bf16; pool layout so casts
  and z-transposes never wait on preamble tile frees.
- Tail: 12 per-head attn transposes batched into 2 six-head xbar
  transposes (each xbar op costs a fixed ~10.4us on the sync queue).
"""

import os
from contextlib import ExitStack

import numpy as np

import concourse.bass as bass
import concourse.bacc as bacc
import concourse.tile as tile
import concourse.mybir as mybir
from concourse.masks import make_identity

S = 1024
CS = 384
CZ = 128
D = 32
H = 12
NCORES = 8
RB = S // NCORES  # 128 query rows per core
JT = S // 128     # 8 column tiles
CKS = CS // 128   # 3 contraction chunks of s-dim
EPS = 1e-5
INVD = 1.0 / np.sqrt(D)

F32 = mybir.dt.float32
BF16 = mybir.dt.bfloat16
F16 = mybir.dt.float16
I32 = mybir.dt.int32
AF = mybir.ActivationFunctionType
OP = mybir.AluOpType


def _mm(nc, out, lhsT, rhs, start, stop, **kw):
    nc.tensor.matmul(out, lhsT, rhs, start=start, stop=stop, **kw)


def build(nc):
    s_full = nc.dram_tensor("s", [S, CS], F32, kind="ExternalInput").ap()
    s_loc = nc.dram_tensor("s_loc", [RB, CS], F32, kind="ExternalInput").ap()
    z_d = nc.dram_tensor("z", [RB, S, CZ], F32, kind="ExternalInput").ap()
    zm_d = nc.dram_tensor("z_mask", [RB, S], I32, kind="ExternalInput").ap()
    ws_d = nc.dram_tensor("w_s", [CS], F32, kind="ExternalInput").ap()
    wz_d = nc.dram_tensor("w_z", [CZ], F32, kind="ExternalInput").ap()
    Wz_d = nc.dram_tensor("Wz", [CZ, H], F32, kind="ExternalInput").ap()
    Wq_d = nc.dram_tensor("Wq", [CS, CS], F32, kind="ExternalInput").ap()
    Wk_d = nc.dram_tensor("Wk", [CS, CS], F32, kind="ExternalInput").ap()
    Wv_d = nc.dram_tensor("Wv", [CS, CS], F32, kind="ExternalInput").ap()
    Wg_d = nc.dram_tensor("Wg", [CS, CS], F32, kind="ExternalInput").ap()
    bg_d = nc.dram_tensor("bg", [CS], F32, kind="ExternalInput").ap()
    Wo_d = nc.dram_tensor("Wo", [CS, CS], F32, kind="ExternalInput").ap()
    bo_d = nc.dram_tensor("bo", [CS], F32, kind="ExternalInput").ap()
    out_d = nc.dram_tensor("out", [RB, CS], F32, kind="ExternalOutput").ap()

    with tile.TileContext(nc) as tc, ExitStack() as ctx:
        sg = ctx.enter_context(tc.tile_pool(name="singles", bufs=1))
        # ---- pools that must never wait on preamble frees: casts + zT ----
        zctx = ExitStack()
        znp = zctx.enter_context(tc.tile_pool(name="znat", bufs=2))
        ztp = zctx.enter_context(tc.tile_pool(name="znT", bufs=2))

        zn_tiles = {}

        def issue_cast(jt):
            znI = znp.tile([128, 128, CZ], BF16, tag="zn", name="znI")
            nc.gpsimd.dma_start(out=znI, in_=z_d[:, bass.ts(jt, 128), :])
            zn_tiles[jt] = znI

        issue_cast(0)
        issue_cast(1)

        # ---------- constants ----------
        ident_f = sg.tile([128, 128], F32)
        make_identity(nc, ident_f)
        ident_b = sg.tile([128, 128], BF16)
        make_identity(nc, ident_b)
        ones1 = sg.tile([1, 128], F32)
        nc.vector.memset(ones1, 1.0)
        eps_t = sg.tile([128, 1], F32)
        nc.vector.memset(eps_t, EPS)

        # small weight-ish loads on the scalar HWDGE ring
        Wz_sb = sg.tile([128, H], F32)
        nc.scalar.dma_start(out=Wz_sb, in_=Wz_d)
        ws_sb = sg.tile([128, CKS], F32)
        nc.scalar.dma_start(out=ws_sb, in_=ws_d.rearrange("(k p) -> p k", p=128))
        wzv_sb = sg.tile([128, 1], F32)
        nc.scalar.dma_start(out=wzv_sb, in_=wz_d.rearrange("(p o) -> p o", o=1))
        bg_sb = sg.tile([1, CS], F32)
        nc.scalar.dma_start(out=bg_sb, in_=bg_d.rearrange("(o c) -> o c", o=1))
        bo_sb = sg.tile([1, CS], F32)
        nc.scalar.dma_start(out=bo_sb, in_=bo_d.rearrange("(o c) -> o c", o=1))
        mask_bf = sg.tile([128, S], BF16)

        # fold w_z into Wz rows -> bf16 stationary-side moving operand
        nc.vector.tensor_scalar_mul(Wz_sb, Wz_sb, wzv_sb)
        Wz_bf = sg.tile([128, H], BF16)
        nc.vector.tensor_copy(out=Wz_bf, in_=Wz_sb)

        # ---------- preamble: rmsnorm(s), q/k/v/g (bf16) ----------
        kT = sg.tile([128, CKS, S], BF16)    # [hd_in_chunk, chunk, j]
        v_sb = sg.tile([128, JT, CS], BF16)  # [j_in_tile, jt, hd]
        qT = sg.tile([128, CKS, 128], BF16)  # [hd_in_chunk, chunk, i_loc]
        g_sb = sg.tile([128, CS], F32)

        with tc.tile_pool(name="pre", bufs=1) as pre, \
             tc.tile_pool(name="pre_tmp", bufs=3) as pt, \
             tc.tile_pool(name="pre_ps", bufs=2, space="PSUM") as pp:
            def norm_rows(ap, out_bf):
                sq = pt.tile([128, CS], BF16, tag="nsq")
                msum = pt.tile([128, 1], F32, tag="nms")
                nc.scalar.activation(out=sq, in_=ap, func=AF.Square,
                                     scale=float(1.0 / np.sqrt(CS)),
                                     accum_out=msum)
                nc.scalar.activation(out=msum, in_=msum, func=AF.Sqrt,
                                     bias=eps_t, scale=1.0)
                nc.vector.reciprocal(out=msum, in_=msum)
                nc.vector.scalar_tensor_tensor(
                    out=out_bf, in0=ap, scalar=1.0,
                    in1=bass.AP(tensor=msum.tensor, offset=msum.offset,
                                ap=[msum.ap[0], [0, CS]]),
                    op0=OP.mult, op1=OP.mult)

            # normalized s in bf16, chunked through a small f32 temp
            s_rb = pre.tile([128, JT, CS], BF16)
            for t in range(JT):
                s_tmp = pt.tile([128, CS], F32, tag="s_tmp")
                nc.scalar.dma_start(
                    out=s_tmp,
                    in_=s_full.rearrange("(t p) c -> p t c", p=128)[:, t, :])
                norm_rows(s_tmp, s_rb[:, t, :])
            s_rlb = pre.tile([128, CS], BF16)
            s_tmp = pt.tile([128, CS], F32, tag="s_tmp")
            nc.scalar.dma_start(out=s_tmp, in_=s_loc)
            norm_rows(s_tmp, s_rlb)

            # transposes of normalized s (bf16)
            s_rT = pre.tile([128, CKS, S], BF16)    # [c, k, i]
            s_rTl = pre.tile([128, CKS, 128], BF16)  # [c, k, local i]
            for t in range(JT):
                for k in range(CKS):
                    ps = pp.tile([128, 128], BF16, tag="tp")
                    _mm(nc, ps, s_rb[:, t, bass.ts(k, 128)], ident_b, True, True,
                        is_transpose=True)
                    nc.scalar.copy(out=s_rT[:, k, bass.ts(t, 128)], in_=ps)
            for k in range(CKS):
                ps = pp.tile([128, 128], BF16, tag="tp")
                _mm(nc, ps, s_rlb[:, bass.ts(k, 128)], ident_b, True, True,
                    is_transpose=True)
                nc.scalar.copy(out=s_rTl[:, k, :], in_=ps)

            # weights one at a time through a rotating f32 temp + bf16 slot
            def load_w(dram):
                wtmp = pre.tile([128, CKS, CS], F32, tag="wtmp", name="wtmp")
                nc.scalar.dma_start(
                    out=wtmp, in_=dram.rearrange("(k p) c -> p k c", p=128))
                wb = pre.tile([128, CKS, CS], BF16, tag="wb", name="wb")
                for k in range(CKS):
                    nc.vector.tensor_scalar_mul(
                        wtmp[:, k, :], wtmp[:, k, :], ws_sb[:, k:k + 1])
                nc.vector.tensor_copy(out=wb, in_=wtmp)
                return wb

            wb = load_w(Wq_d)
            for k in range(CKS):
                ps = pp.tile([128, 128], F32, tag="qp")
                for ck in range(CKS):
                    _mm(nc, ps, wb[:, ck, bass.ts(k, 128)],
                        s_rTl[:, ck, :], ck == 0, ck == CKS - 1)
                nc.scalar.mul(out=qT[:, k, :], in_=ps, mul=float(INVD))
            wb = load_w(Wk_d)
            for k in range(CKS):
                for half in range(2):
                    ps2 = pp.tile([128, 512], F32, tag="big")
                    for ck in range(CKS):
                        _mm(nc, ps2, wb[:, ck, bass.ts(k, 128)],
                            s_rT[:, ck, bass.ts(half, 512)], ck == 0, ck == CKS - 1)
                    nc.scalar.copy(out=kT[:, k, bass.ts(half, 512)], in_=ps2)
            wb = load_w(Wv_d)
            for jc in range(JT):
                ps2 = pp.tile([128, 512], F32, tag="big")
                for ck in range(CKS):
                    _mm(nc, ps2[:, 0:CS], s_rT[:, ck, bass.ts(jc, 128)],
                        wb[:, ck, :], ck == 0, ck == CKS - 1)
                nc.scalar.copy(out=v_sb[:, jc, :], in_=ps2[:, 0:CS])
            wb = load_w(Wg_d)
            ps2 = pp.tile([128, 512], F32, tag="big")
            for ck in range(CKS):
                _mm(nc, ps2[:, 0:CS], s_rTl[:, ck, :], wb[:, ck, :],
                    ck == 0, False)
            _mm(nc, ps2[:, 0:CS], ones1, bg_sb, False, True)
            nc.scalar.copy(out=g_sb, in_=ps2[:, 0:CS])

        # scores staging, allocated after the preamble pool closes
        sc_st = sg.tile([128, H, JT, 128], F16)       # [i, h, jt, j]

        # ---------- z stream: jt-major ----------
        BI = 32
        NB = RB // BI               # 4 batches of 32 j per jt

        with tc.tile_pool(name="sqp", bufs=2) as sqp, \
             tc.tile_pool(name="trp", bufs=1) as trp, \
             tc.tile_pool(name="msp", bufs=2) as msp, \
             tc.tile_pool(name="bjt", bufs=2) as bjp, \
             tc.tile_pool(name="bias_ps", bufs=2, space="PSUM") as bpp, \
             tc.tile_pool(name="sc_ps", bufs=4, space="PSUM") as scp:

            def emit_scores(jt, B_jt):
                for h in range(H):
                    ck, hp = divmod(h, 4)
                    sc = scp.tile([128, 128], F32, tag="sc", name="sc")
                    _mm(nc, sc, qT[bass.ts(hp, 32), ck, :],
                        kT[bass.ts(hp, 32), ck, bass.ts(jt, 128)],
                        True, True, tile_position=(32 * hp, 0))
                    b_slice = bass.AP(
                        tensor=B_jt.tensor,
                        offset=B_jt.offset + h,
                        ap=[B_jt.ap[0], [H, 128]])
                    nc.vector.scalar_tensor_tensor(
                        out=sc_st[:, h, jt, :], in0=sc, scalar=1.0,
                        in1=b_slice, op0=OP.mult, op1=OP.add)

            pend_jt = None
            for jt in range(JT):
                znI = zn_tiles.pop(jt)
                zt = ztp.tile([128, 128, 128], BF16, tag="zt", name="zt")
                nc.sync.dma_start(out=zt, in_=znI, transpose=True)

                # stats: squares (ScalarE/GpSimd) + DVE bf16 halving tree
                msI = msp.tile([128, 128], F32, tag="msI", name="msI")
                for q in range(4):
                    sq = sqp.tile([128, BI, CZ], BF16, tag="sq", name="sq")
                    src = znI[:, bass.ts(q, BI), :]
                    if q == 3:
                        nc.gpsimd.tensor_mul(sq, src, src)
                    else:
                        nc.scalar.square(out=sq, in_=src)
                    tA = trp.tile([128, BI, 64], BF16, tag="tA")
                    nc.vector.tensor_tensor(
                        out=tA, in0=sq[:, :, 0:64], in1=sq[:, :, 64:128],
                        op=OP.add)
                    tB = trp.tile([128, BI, 32], BF16, tag="tB")
                    nc.vector.tensor_tensor(
                        out=tB, in0=tA[:, :, 0:32], in1=tA[:, :, 32:64],
                        op=OP.add)
                    tC = trp.tile([128, BI, 16], BF16, tag="tC")
                    nc.vector.tensor_tensor(
                        out=tC, in0=tB[:, :, 0:16], in1=tB[:, :, 16:32],
                        op=OP.add)
                    tD = trp.tile([128, BI, 8], BF16, tag="tD")
                    nc.vector.tensor_tensor(
                        out=tD, in0=tC[:, :, 0:8], in1=tC[:, :, 8:16],
                        op=OP.add)
                    nc.vector.tensor_reduce(
                        out=msI[:, bass.ts(q, BI)], in_=tD,
                        axis=mybir.AxisListType.X, op=OP.add)
                # rs = 1/sqrt(ms/CZ + eps), [i, j] orientation
                nc.scalar.activation(out=msI, in_=msI, func=AF.Sqrt,
                                     bias=eps_t, scale=float(1.0 / CZ))
                nc.vector.reciprocal(out=msI, in_=msI)

                # prefetch next cast AFTER the squares in GpSimd FIFO order
                if jt + 2 < JT:
                    issue_cast(jt + 2)

                B_jt = bjp.tile([128, RB, H], BF16, tag="bjt", name="B_jt")
                for b in range(NB):
                    j0 = b * BI
                    # separate banks per col-group so the LDW of one group
                    # overlaps the other group's in-flight matmul
                    b_psA = bpp.tile([128, BI, H], F32, tag="bpsA", name="b_psA")
                    b_psB = bpp.tile([128, BI, H], F32, tag="bpsB", name="b_psB")
                    for jj in range(BI):
                        j = j0 + jj
                        _mm(nc, b_psA[0:64, jj, :], zt[:, j, 0:64], Wz_bf,
                            jj == 0, jj == BI - 1, tile_position=(0, 0))
                        _mm(nc, b_psB[64:128, jj, :], zt[:, j, 64:128], Wz_bf,
                            jj == 0, jj == BI - 1, tile_position=(0, 64))
                    for lo, hi, bp in ((0, 64, b_psA), (64, 128, b_psB)):
                        msl = msI[lo:hi, :]
                        rs_b = bass.AP(
                            tensor=msl.tensor,
                            offset=msl.offset + j0,
                            ap=[msl.ap[0], [1, BI], [0, H]])
                        nc.vector.tensor_tensor(
                            out=B_jt[lo:hi, j0:j0 + BI, :],
                            in0=bp[lo:hi, :, :], in1=rs_b,
                            op=OP.mult)
                if pend_jt is not None:
                    emit_scores(*pend_jt)
                pend_jt = (jt, B_jt)
            emit_scores(*pend_jt)
        zctx.close()

        # ---------- attention tail ----------
        HB = 6   # heads per transpose batch
        with tc.tile_pool(name="att_sb", bufs=2) as asb, \
             tc.tile_pool(name="attT_sb", bufs=2) as atsb, \
             tc.tile_pool(name="den_sb", bufs=2) as dsb, \
             tc.tile_pool(name="tailw", bufs=1) as tlw, \
             tc.tile_pool(name="o_ps", bufs=2, space="PSUM") as opp, \
             tc.tile_pool(name="fin_ps", bufs=1, space="PSUM") as fpp:
            mi = tlw.tile([128, S], I32)
            nc.scalar.dma_start(out=mi, in_=zm_d)
            nc.vector.tensor_copy(out=mask_bf, in_=mi)
            wo_f = tlw.tile([128, CKS, CS], F32, name="w_Wo")
            nc.scalar.dma_start(
                out=wo_f, in_=Wo_d.rearrange("(k p) c -> p k c", p=128))
            oT_sb = sg.tile([128, CKS, 128], F32)   # [hd_in_chunk, chunk, i]
            for batch in range(H // HB):
                attb = asb.tile([128, HB, JT, 128], BF16, tag="attb",
                                name="attb")
                for hh in range(HB):
                    h = batch * HB + hh
                    att = attb[:, hh, :, :]
                    nc.scalar.activation(out=att, in_=sc_st[:, h, :, :],
                                         func=AF.Exp)
                    den = dsb.tile([128, 1], F32, tag="den", name="den")
                    nc.vector.scalar_tensor_tensor(
                        out=att, in0=att, scalar=1.0, in1=mask_bf,
                        op0=OP.mult, op1=OP.mult, accum_out=den)
                    nc.vector.reciprocal(out=den, in_=den)
                    nc.vector.tensor_scalar_mul(att, att, den)
                atT = atsb.tile([128, HB, JT, 128], BF16, tag="atT",
                                name="atT")
                nc.sync.dma_start(out=atT, in_=attb, transpose=True)
                for hh in range(HB):
                    h = batch * HB + hh
                    o_ps = opp.tile([32, 128], F32, tag="o", name="o_ps")
                    for jc in range(JT):
                        _mm(nc, o_ps, v_sb[:, jc, bass.ts(h, 32)],
                            atT[:, hh, jc, :], jc == 0, jc == JT - 1)
                    ck, hp = divmod(h, 4)
                    nc.scalar.copy(out=oT_sb[bass.ts(hp, 32), ck, :], in_=o_ps)

            # ---------- output ----------
            fin = fpp.tile([128, CS], F32, tag="fin")
            for k in range(CKS):
                _mm(nc, fin[:, 0:CS], oT_sb[:, k, :], wo_f[:, k, :],
                    k == 0, False)
            _mm(nc, fin[:, 0:CS], ones1, bo_sb, False, True)
            out_sb = sg.tile([128, CS], F32)
            nc.vector.tensor_tensor(out=out_sb, in0=fin[:, 0:CS], in1=g_sb,
                                    op=OP.mult)
            nc.sync.dma_start(out=out_d, in_=out_sb)

    nc.compile()
    return nc


_NC_CACHE = None


def _get_nc():
    global _NC_CACHE
    if _NC_CACHE is None:
        nc = bacc.Bacc("TRN2", target_bir_lowering=False, debug=False,
                       enable_asserts=False)
        _NC_CACHE = build(nc)
    return _NC_CACHE


def make_in_maps(s, z, z_mask, w_s, w_z, Wz, Wq, Wk, Wv, Wg, bg, Wo, bo):
    f = lambda a: np.ascontiguousarray(np.asarray(a), dtype=np.float32)
    s = f(s)
    shared = dict(s=s, w_s=f(w_s), w_z=f(w_z), Wz=f(Wz), Wq=f(Wq), Wk=f(Wk),
                  Wv=f(Wv), Wg=f(Wg), bg=f(bg), Wo=f(Wo), bo=f(bo))
    zmask = np.ascontiguousarray(np.asarray(z_mask), dtype=np.int32)
    z = f(z)
    in_maps = []
    for c in range(NCORES):
        r0, r1 = c * RB, (c + 1) * RB
        m = dict(shared)
        m["s_loc"] = np.ascontiguousarray(s[r0:r1])
        m["z"] = np.ascontiguousarray(z[r0:r1])
        m["z_mask"] = np.ascontiguousarray(zmask[r0:r1])
        in_maps.append(m)
    return in_maps


def kernel(**inputs):
    from concourse import bass_utils
    nc = _get_nc()
    in_maps = make_in_maps(**inputs)
    res = bass_utils.run_bass_kernel_spmd(nc, in_maps, core_ids=list(range(NCORES)))
    out = np.concatenate([res.results[c]["out"] for c in range(NCORES)], axis=0)
    return out.astype(np.float32)


# revision 24
# speedup vs baseline: 1.0769x; 1.0769x over previous
"""AttentionWithPairBias distributed Trainium2 kernel (8 NeuronCores).

Sequence-parallel: core c owns query rows i in [128c, 128(c+1)).
Per core: z shard [128, 1024, 128] f32 (64MB -> memory roofline), s and
weights replicated. No collectives.

v2 changes over the 507us baseline (trace-driven):
- Bias matmuls 2-way column-tiled (i split 64+64, tile_position (0,0)
  and (0,64)): LDWEIGHTS of one col-group overlaps the other group's
  in-flight matmul, breaking the LDW->MM drain serialization seen in
  the trace (104+172ns per j).
- z^2 stats: DVE bf16 tensor_tensor halving-tree (2x mode) replaces
  the 1x tensor_reduce (146us). Squares split ScalarE/GpSimd.
- cast prefetch DGE issued after the squares in program order so the
  GpSimd FIFO doesn't stall squares behind the descriptor-gen that
  waits on the transpose (trace showed strict cast/transpose
  alternation from this chain).
- Preamble DMAs moved to the scalar HWDGE ring; sync ring carries only
  the 8 z transposes. Preamble weights/s in bf16; pool layout so casts
  and z-transposes never wait on preamble tile frees.
- Tail: 12 per-head attn transposes batched into 2 six-head xbar
  transposes (each xbar op costs a fixed ~10.4us on the sync queue).
"""

import os
from contextlib import ExitStack

import numpy as np

import concourse.bass as bass
import concourse.bacc as bacc
import concourse.tile as tile
import concourse.mybir as mybir
from concourse.masks import make_identity

S = 1024
CS = 384
CZ = 128
D = 32
H = 12
NCORES = 8
RB = S // NCORES  # 128 query rows per core
JT = S // 128     # 8 column tiles
CKS = CS // 128   # 3 contraction chunks of s-dim
EPS = 1e-5
INVD = 1.0 / np.sqrt(D)

F32 = mybir.dt.float32
BF16 = mybir.dt.bfloat16
F16 = mybir.dt.float16
I32 = mybir.dt.int32
AF = mybir.ActivationFunctionType
OP = mybir.AluOpType


def _mm(nc, out, lhsT, rhs, start, stop, **kw):
    nc.tensor.matmul(out, lhsT, rhs, start=start, stop=stop, **kw)


def build(nc):
    s_full = nc.dram_tensor("s", [S, CS], F32, kind="ExternalInput").ap()
    s_loc = nc.dram_tensor("s_loc", [RB, CS], F32, kind="ExternalInput").ap()
    z_d = nc.dram_tensor("z", [RB, S, CZ], F32, kind="ExternalInput").ap()
    zm_d = nc.dram_tensor("z_mask", [RB, S], I32, kind="ExternalInput").ap()
    ws_d = nc.dram_tensor("w_s", [CS], F32, kind="ExternalInput").ap()
    wz_d = nc.dram_tensor("w_z", [CZ], F32, kind="ExternalInput").ap()
    Wz_d = nc.dram_tensor("Wz", [CZ, H], F32, kind="ExternalInput").ap()
    Wq_d = nc.dram_tensor("Wq", [CS, CS], F32, kind="ExternalInput").ap()
    Wk_d = nc.dram_tensor("Wk", [CS, CS], F32, kind="ExternalInput").ap()
    Wv_d = nc.dram_tensor("Wv", [CS, CS], F32, kind="ExternalInput").ap()
    Wg_d = nc.dram_tensor("Wg", [CS, CS], F32, kind="ExternalInput").ap()
    bg_d = nc.dram_tensor("bg", [CS], F32, kind="ExternalInput").ap()
    Wo_d = nc.dram_tensor("Wo", [CS, CS], F32, kind="ExternalInput").ap()
    bo_d = nc.dram_tensor("bo", [CS], F32, kind="ExternalInput").ap()
    out_d = nc.dram_tensor("out", [RB, CS], F32, kind="ExternalOutput").ap()

    with tile.TileContext(nc) as tc, ExitStack() as ctx:
        sg = ctx.enter_context(tc.tile_pool(name="singles", bufs=1))
        # ---- pools that must never wait on preamble frees: casts + zT ----
        zctx = ExitStack()
        znp = zctx.enter_context(tc.tile_pool(name="znat", bufs=2))
        ztp = zctx.enter_context(tc.tile_pool(name="znT", bufs=2))
        sqp = zctx.enter_context(tc.tile_pool(name="sqp", bufs=2))
        trp = zctx.enter_context(tc.tile_pool(name="trp", bufs=1))
        msp = zctx.enter_context(tc.tile_pool(name="msp", bufs=2))

        zn_tiles = {}

        def issue_cast(jt):
            znI = znp.tile([128, 128, CZ], BF16, tag="zn", name="znI")
            nc.gpsimd.dma_start(out=znI, in_=z_d[:, bass.ts(jt, 128), :])
            zn_tiles[jt] = znI

        issue_cast(0)
        issue_cast(1)

        # ---------- constants ----------
        ident_b = sg.tile([128, 128], BF16)
        make_identity(nc, ident_b)
        ones1 = sg.tile([1, 128], F32)
        nc.vector.memset(ones1, 1.0)
        eps_t = sg.tile([128, 1], F32)
        nc.vector.memset(eps_t, EPS)

        # stats for one column tile: squares (ScalarE / DVE split) + bf16
        # halving tree -> msI[i, j] = 1/sqrt(mean_c z^2 + eps)
        BI = 32
        SB = 16

        def stats_jt(znI):
            msI = msp.tile([128, 128], F32, tag="msI", name="msI")
            for q in range(8):
                sq = sqp.tile([128, SB, CZ], BF16, tag="sq", name="sq")
                src = znI[:, bass.ts(q, SB), :]
                if q < 4:
                    nc.scalar.square(out=sq, in_=src)
                else:
                    nc.vector.tensor_tensor(out=sq, in0=src, in1=src,
                                            op=OP.mult)
                tA = trp.tile([128, SB, 64], BF16, tag="tA")
                nc.vector.tensor_tensor(
                    out=tA, in0=sq[:, :, 0:64], in1=sq[:, :, 64:128], op=OP.add)
                tB = trp.tile([128, SB, 32], BF16, tag="tB")
                nc.vector.tensor_tensor(
                    out=tB, in0=tA[:, :, 0:32], in1=tA[:, :, 32:64], op=OP.add)
                tC = trp.tile([128, SB, 16], BF16, tag="tC")
                nc.vector.tensor_tensor(
                    out=tC, in0=tB[:, :, 0:16], in1=tB[:, :, 16:32], op=OP.add)
                tD = trp.tile([128, SB, 8], BF16, tag="tD")
                nc.vector.tensor_tensor(
                    out=tD, in0=tC[:, :, 0:8], in1=tC[:, :, 8:16], op=OP.add)
                nc.vector.tensor_reduce(
                    out=msI[:, bass.ts(q, SB)], in_=tD,
                    axis=mybir.AxisListType.X, op=OP.add)
            nc.scalar.activation(out=msI, in_=msI, func=AF.Sqrt,
                                 bias=eps_t, scale=float(1.0 / CZ))
            nc.vector.reciprocal(out=msI, in_=msI)
            return msI

        # prime jt 0/1 stats ahead of the preamble so they sit early in
        # the ScalarE/DVE FIFOs (preamble work otherwise blocks bias(0))
        ms_tiles = {0: stats_jt(zn_tiles[0]), 1: stats_jt(zn_tiles[1])}

        # small weight-ish loads on the scalar HWDGE ring
        Wz_sb = sg.tile([128, H], F32)
        nc.scalar.dma_start(out=Wz_sb, in_=Wz_d)
        ws_sb = sg.tile([128, CKS], F32)
        nc.scalar.dma_start(out=ws_sb, in_=ws_d.rearrange("(k p) -> p k", p=128))
        wzv_sb = sg.tile([128, 1], F32)
        nc.scalar.dma_start(out=wzv_sb, in_=wz_d.rearrange("(p o) -> p o", o=1))
        bg_sb = sg.tile([1, CS], F32)
        nc.scalar.dma_start(out=bg_sb, in_=bg_d.rearrange("(o c) -> o c", o=1))
        bo_sb = sg.tile([1, CS], F32)
        nc.scalar.dma_start(out=bo_sb, in_=bo_d.rearrange("(o c) -> o c", o=1))

        # fold w_z into Wz rows -> bf16 stationary-side moving operand
        nc.vector.tensor_scalar_mul(Wz_sb, Wz_sb, wzv_sb)
        Wz_bf = sg.tile([128, H], BF16)
        nc.vector.tensor_copy(out=Wz_bf, in_=Wz_sb)

        # ---------- preamble: rmsnorm(s), q/k/v/g (bf16) ----------
        kT = sg.tile([128, CKS, S], BF16)    # [hd_in_chunk, chunk, j]
        v_sb = sg.tile([128, JT, CS], BF16)  # [j_in_tile, jt, hd]
        qT = sg.tile([128, CKS, 128], BF16)  # [hd_in_chunk, chunk, i_loc]
        g_sb = sg.tile([128, CS], F32)

        with tc.tile_pool(name="pre", bufs=1) as pre, \
             tc.tile_pool(name="pre_tmp", bufs=2) as pt, \
             tc.tile_pool(name="pre_ps", bufs=2, space="PSUM") as pp:
            def norm_rows(ap, out_bf):
                sq = pt.tile([128, CS], BF16, tag="nsq")
                msum = pt.tile([128, 1], F32, tag="nms")
                nc.scalar.activation(out=sq, in_=ap, func=AF.Square,
                                     scale=float(1.0 / np.sqrt(CS)),
                                     accum_out=msum)
                nc.scalar.activation(out=msum, in_=msum, func=AF.Sqrt,
                                     bias=eps_t, scale=1.0)
                nc.vector.reciprocal(out=msum, in_=msum)
                nc.vector.scalar_tensor_tensor(
                    out=out_bf, in0=ap, scalar=1.0,
                    in1=bass.AP(tensor=msum.tensor, offset=msum.offset,
                                ap=[msum.ap[0], [0, CS]]),
                    op0=OP.mult, op1=OP.mult)

            # normalized s in bf16, chunked through a small f32 temp
            s_rb = pre.tile([128, JT, CS], BF16)
            for t in range(JT):
                s_tmp = pt.tile([128, CS], F32, tag="s_tmp")
                nc.scalar.dma_start(
                    out=s_tmp,
                    in_=s_full.rearrange("(t p) c -> p t c", p=128)[:, t, :])
                norm_rows(s_tmp, s_rb[:, t, :])
            s_rlb = pre.tile([128, CS], BF16)
            s_tmp = pt.tile([128, CS], F32, tag="s_tmp")
            nc.scalar.dma_start(out=s_tmp, in_=s_loc)
            norm_rows(s_tmp, s_rlb)

            # transposes of normalized s (bf16)
            s_rT = pre.tile([128, CKS, S], BF16)    # [c, k, i]
            s_rTl = pre.tile([128, CKS, 128], BF16)  # [c, k, local i]
            ncopy = 0

            def drain(dst, src):
                nonlocal ncopy
                if ncopy % 2 == 0:
                    nc.vector.tensor_copy(out=dst, in_=src)
                else:
                    nc.scalar.copy(out=dst, in_=src)
                ncopy += 1

            for t in range(JT):
                for k in range(CKS):
                    ps = pp.tile([128, 128], BF16, tag="tp")
                    _mm(nc, ps, s_rb[:, t, bass.ts(k, 128)], ident_b, True, True,
                        is_transpose=True)
                    drain(s_rT[:, k, bass.ts(t, 128)], ps)
            for k in range(CKS):
                ps = pp.tile([128, 128], BF16, tag="tp")
                _mm(nc, ps, s_rlb[:, bass.ts(k, 128)], ident_b, True, True,
                    is_transpose=True)
                drain(s_rTl[:, k, :], ps)

            # w_s folded into s^T (per-partition scalar after transpose),
            # so weights go straight to bf16 via SWDGE cast DMA
            for k in range(CKS):
                nc.vector.tensor_scalar_mul(
                    s_rT[:, k, :], s_rT[:, k, :], ws_sb[:, k:k + 1])
                nc.vector.tensor_scalar_mul(
                    s_rTl[:, k, :], s_rTl[:, k, :], ws_sb[:, k:k + 1])

            def load_w(dram):
                wb = pre.tile([128, CKS, CS], BF16, tag="wb", name="wb")
                nc.gpsimd.dma_start(
                    out=wb, in_=dram.rearrange("(k p) c -> p k c", p=128))
                return wb

            wb = load_w(Wq_d)
            for k in range(CKS):
                ps = pp.tile([128, 128], F32, tag="qp")
                for ck in range(CKS):
                    _mm(nc, ps, wb[:, ck, bass.ts(k, 128)],
                        s_rTl[:, ck, :], ck == 0, ck == CKS - 1)
                nc.scalar.mul(out=qT[:, k, :], in_=ps, mul=float(INVD))
            wb = load_w(Wk_d)
            for k in range(CKS):
                for half in range(2):
                    ps2 = pp.tile([128, 512], F32, tag="big")
                    for ck in range(CKS):
                        _mm(nc, ps2, wb[:, ck, bass.ts(k, 128)],
                            s_rT[:, ck, bass.ts(half, 512)], ck == 0, ck == CKS - 1)
                    drain(kT[:, k, bass.ts(half, 512)], ps2)
            wb = load_w(Wv_d)
            for jc in range(JT):
                ps2 = pp.tile([128, 512], F32, tag="big")
                for ck in range(CKS):
                    _mm(nc, ps2[:, 0:CS], s_rT[:, ck, bass.ts(jc, 128)],
                        wb[:, ck, :], ck == 0, ck == CKS - 1)
                drain(v_sb[:, jc, :], ps2[:, 0:CS])
            wb = load_w(Wg_d)
            ps2 = pp.tile([128, 512], F32, tag="big")
            for ck in range(CKS):
                _mm(nc, ps2[:, 0:CS], s_rTl[:, ck, :], wb[:, ck, :],
                    ck == 0, False)
            _mm(nc, ps2[:, 0:CS], ones1, bg_sb, False, True)
            nc.scalar.copy(out=g_sb, in_=ps2[:, 0:CS])

        # scores staging, allocated after the preamble pool closes
        sc_st = sg.tile([128, H, JT, 128], F16)       # [i, h, jt, j]

        # ---------- z stream: jt-major ----------
        NB = RB // BI               # 4 batches of 32 j per jt

        with tc.tile_pool(name="bjt", bufs=2) as bjp, \
             tc.tile_pool(name="bias_ps", bufs=4, space="PSUM") as bpp, \
             tc.tile_pool(name="sc_ps", bufs=4, space="PSUM") as scp:

            def emit_scores(jt, B_jt):
                for h in range(H):
                    ck, hp = divmod(h, 4)
                    sc = scp.tile([128, 128], F32, tag="sc", name="sc")
                    _mm(nc, sc, qT[bass.ts(hp, 32), ck, :],
                        kT[bass.ts(hp, 32), ck, bass.ts(jt, 128)],
                        True, True, tile_position=(32 * hp, 0))
                    b_slice = bass.AP(
                        tensor=B_jt.tensor,
                        offset=B_jt.offset + h,
                        ap=[B_jt.ap[0], [H, 128]])
                    nc.vector.scalar_tensor_tensor(
                        out=sc_st[:, h, jt, :], in0=sc, scalar=1.0,
                        in1=b_slice, op0=OP.mult, op1=OP.add)

            pend_jt = None
            for jt in range(JT):
                znI = zn_tiles.pop(jt)
                zt = ztp.tile([128, 128, 128], BF16, tag="zt", name="zt")
                nc.sync.dma_start(out=zt, in_=znI, transpose=True)

                msI = ms_tiles.pop(jt) if jt in ms_tiles else stats_jt(znI)

                if jt + 2 < JT:
                    issue_cast(jt + 2)

                B_jt = bjp.tile([128, RB, H], BF16, tag="bjt", name="B_jt")
                for b in range(NB):
                    j0 = b * BI
                    b_ps = bpp.tile([128, BI, H], F32, tag="bps", name="b_ps")
                    for jj in range(BI):
                        _mm(nc, b_ps[:, jj, :], zt[:, j0 + jj, :], Wz_bf,
                            jj == 0, jj == BI - 1)
                    rs_b = bass.AP(
                        tensor=msI.tensor,
                        offset=msI.offset + j0,
                        ap=[msI.ap[0], [1, BI], [0, H]])
                    nc.vector.tensor_tensor(
                        out=B_jt[:, j0:j0 + BI, :], in0=b_ps, in1=rs_b,
                        op=OP.mult)
                if pend_jt is not None:
                    emit_scores(*pend_jt)
                pend_jt = (jt, B_jt)
            emit_scores(*pend_jt)
        zctx.close()

        # ---------- attention tail ----------
        HB = 6   # heads per transpose batch
        with tc.tile_pool(name="att_sb", bufs=2) as asb, \
             tc.tile_pool(name="attT_sb", bufs=2) as atsb, \
             tc.tile_pool(name="den_sb", bufs=2) as dsb, \
             tc.tile_pool(name="tailw", bufs=1) as tlw, \
             tc.tile_pool(name="o_ps", bufs=2, space="PSUM") as opp, \
             tc.tile_pool(name="fin_ps", bufs=1, space="PSUM") as fpp:
            mask_bf = tlw.tile([128, S], BF16)
            mi = tlw.tile([128, S], I32)
            nc.scalar.dma_start(out=mi, in_=zm_d)
            nc.vector.tensor_copy(out=mask_bf, in_=mi)
            wo_f = tlw.tile([128, CKS, CS], F32, name="w_Wo")
            nc.scalar.dma_start(
                out=wo_f, in_=Wo_d.rearrange("(k p) c -> p k c", p=128))
            oT_sb = sg.tile([128, CKS, 128], F32)   # [hd_in_chunk, chunk, i]
            for batch in range(H // HB):
                attb = asb.tile([128, HB, JT, 128], BF16, tag="attb",
                                name="attb")
                for hh in range(HB):
                    h = batch * HB + hh
                    att = attb[:, hh, :, :]
                    nc.scalar.activation(out=att, in_=sc_st[:, h, :, :],
                                         func=AF.Exp)
                    den = dsb.tile([128, 1], F32, tag="den", name="den")
                    nc.vector.scalar_tensor_tensor(
                        out=att, in0=att, scalar=1.0, in1=mask_bf,
                        op0=OP.mult, op1=OP.mult, accum_out=den)
                    nc.vector.reciprocal(out=den, in_=den)
                    nc.vector.tensor_scalar_mul(att, att, den)
                atT = atsb.tile([128, HB, JT, 128], BF16, tag="atT",
                                name="atT")
                nc.sync.dma_start(out=atT, in_=attb, transpose=True)
                for hh in range(HB):
                    h = batch * HB + hh
                    o_ps = opp.tile([32, 128], F32, tag="o", name="o_ps")
                    for jc in range(JT):
                        _mm(nc, o_ps, v_sb[:, jc, bass.ts(h, 32)],
                            atT[:, hh, jc, :], jc == 0, jc == JT - 1)
                    ck, hp = divmod(h, 4)
                    nc.scalar.copy(out=oT_sb[bass.ts(hp, 32), ck, :], in_=o_ps)

            # ---------- output ----------
            fin = fpp.tile([128, CS], F32, tag="fin")
            for k in range(CKS):
                _mm(nc, fin[:, 0:CS], oT_sb[:, k, :], wo_f[:, k, :],
                    k == 0, False)
            _mm(nc, fin[:, 0:CS], ones1, bo_sb, False, True)
            out_sb = sg.tile([128, CS], F32)
            nc.vector.tensor_tensor(out=out_sb, in0=fin[:, 0:CS], in1=g_sb,
                                    op=OP.mult)
            nc.sync.dma_start(out=out_d, in_=out_sb)

    nc.compile()
    return nc


_NC_CACHE = None


def _get_nc():
    global _NC_CACHE
    if _NC_CACHE is None:
        nc = bacc.Bacc("TRN2", target_bir_lowering=False, debug=False,
                       enable_asserts=False)
        _NC_CACHE = build(nc)
    return _NC_CACHE


def make_in_maps(s, z, z_mask, w_s, w_z, Wz, Wq, Wk, Wv, Wg, bg, Wo, bo):
    f = lambda a: np.ascontiguousarray(np.asarray(a), dtype=np.float32)
    s = f(s)
    shared = dict(s=s, w_s=f(w_s), w_z=f(w_z), Wz=f(Wz), Wq=f(Wq), Wk=f(Wk),
                  Wv=f(Wv), Wg=f(Wg), bg=f(bg), Wo=f(Wo), bo=f(bo))
    zmask = np.ascontiguousarray(np.asarray(z_mask), dtype=np.int32)
    z = f(z)
    in_maps = []
    for c in range(NCORES):
        r0, r1 = c * RB, (c + 1) * RB
        m = dict(shared)
        m["s_loc"] = np.ascontiguousarray(s[r0:r1])
        m["z"] = np.ascontiguousarray(z[r0:r1])
        m["z_mask"] = np.ascontiguousarray(zmask[r0:r1])
        in_maps.append(m)
    return in_maps


def kernel(**inputs):
    from concourse import bass_utils
    nc = _get_nc()
    in_maps = make_in_maps(**inputs)
    res = bass_utils.run_bass_kernel_spmd(nc, in_maps, core_ids=list(range(NCORES)))
    out = np.concatenate([res.results[c]["out"] for c in range(NCORES)], axis=0)
    return out.astype(np.float32)


# revision 25
# speedup vs baseline: 1.1491x; 1.0670x over previous
"""AttentionWithPairBias distributed Trainium2 kernel (8 NeuronCores).

Sequence-parallel: core c owns query rows i in [128c, 128(c+1)).
Per core: z shard [128, 1024, 128] f32 (64MB -> memory roofline), s and
weights replicated. No collectives.

v4 structure (trace-driven; baseline 507us):
- z-pipe primed before the preamble: transposes + stats + bias matmuls
  for jt 0/1 are emitted first so no engine FIFO puts preamble work in
  front of them (v3 lost ~120us to bias(0) queued behind the preamble).
- Scores (qk) moved out of the z-loop into the tail as N=512 matmuls
  against the persisted per-jt bias tiles B_all; z-loop PE work is pure
  bias matmuls. qk stays f32 in PSUM until the exp.
- z^2 stats: squares ScalarE/DVE split + DVE bf16 halving tree (2x)
  instead of 1x tensor_reduce; stats for jt 2/3 are all-DVE so they
  don't queue behind preamble ScalarE work.
- Preamble: s normalized in row-tile chunks (f32 temp -> bf16), w_s
  folded into s^T after the PE transpose, weights cast f32->bf16 by the
  SWDGE DMA itself, all preamble DMAs on the scalar HWDGE ring; sync
  ring carries only the 8 z transposes (casts are SWDGE).
- Tail: per-head exp/mask/normalize, 2 batched 6-head xbar transposes,
  attn @ v, then (o @ Wo + bo) * g.
"""

import os
from contextlib import ExitStack

import numpy as np

import concourse.bass as bass
import concourse.bacc as bacc
import concourse.tile as tile
import concourse.mybir as mybir
from concourse.masks import make_identity

S = 1024
CS = 384
CZ = 128
D = 32
H = 12
NCORES = 8
RB = S // NCORES  # 128 query rows per core
JT = S // 128     # 8 column tiles
CKS = CS // 128   # 3 contraction chunks of s-dim
EPS = 1e-5
INVD = 1.0 / np.sqrt(D)

F32 = mybir.dt.float32
BF16 = mybir.dt.bfloat16
F16 = mybir.dt.float16
I32 = mybir.dt.int32
AF = mybir.ActivationFunctionType
OP = mybir.AluOpType

BI = 32   # bias psum batch (32*12*4B = 1.5KB <= bank)
SB = 16   # stats chunk (j per square/tree pass)


def _mm(nc, out, lhsT, rhs, start, stop, **kw):
    nc.tensor.matmul(out, lhsT, rhs, start=start, stop=stop, **kw)


def build(nc):
    s_full = nc.dram_tensor("s", [S, CS], F32, kind="ExternalInput").ap()
    s_loc = nc.dram_tensor("s_loc", [RB, CS], F32, kind="ExternalInput").ap()
    z_d = nc.dram_tensor("z", [RB, S, CZ], F32, kind="ExternalInput").ap()
    zm_d = nc.dram_tensor("z_mask", [RB, S], I32, kind="ExternalInput").ap()
    ws_d = nc.dram_tensor("w_s", [CS], F32, kind="ExternalInput").ap()
    wz_d = nc.dram_tensor("w_z", [CZ], F32, kind="ExternalInput").ap()
    Wz_d = nc.dram_tensor("Wz", [CZ, H], F32, kind="ExternalInput").ap()
    Wq_d = nc.dram_tensor("Wq", [CS, CS], F32, kind="ExternalInput").ap()
    Wk_d = nc.dram_tensor("Wk", [CS, CS], F32, kind="ExternalInput").ap()
    Wv_d = nc.dram_tensor("Wv", [CS, CS], F32, kind="ExternalInput").ap()
    Wg_d = nc.dram_tensor("Wg", [CS, CS], F32, kind="ExternalInput").ap()
    bg_d = nc.dram_tensor("bg", [CS], F32, kind="ExternalInput").ap()
    Wo_d = nc.dram_tensor("Wo", [CS, CS], F32, kind="ExternalInput").ap()
    bo_d = nc.dram_tensor("bo", [CS], F32, kind="ExternalInput").ap()
    out_d = nc.dram_tensor("out", [RB, CS], F32, kind="ExternalOutput").ap()

    with tile.TileContext(nc) as tc, ExitStack() as ctx:
        sg = ctx.enter_context(tc.tile_pool(name="singles", bufs=1))
        zctx = ExitStack()
        znp = zctx.enter_context(tc.tile_pool(name="znat", bufs=2))
        ztp = zctx.enter_context(tc.tile_pool(name="znT", bufs=2))
        sqp = zctx.enter_context(tc.tile_pool(name="sqp", bufs=2))
        trp = zctx.enter_context(tc.tile_pool(name="trp", bufs=1))
        msp = zctx.enter_context(tc.tile_pool(name="msp", bufs=2))
        bpp = zctx.enter_context(tc.tile_pool(name="bias_ps", bufs=4,
                                              space="PSUM"))

        zn_tiles = {}

        def issue_cast(jt):
            znI = znp.tile([128, 128, CZ], BF16, tag="zn", name="znI")
            nc.gpsimd.dma_start(out=znI, in_=z_d[:, bass.ts(jt, 128), :])
            zn_tiles[jt] = znI

        issue_cast(0)
        issue_cast(1)

        # ---------- constants / tiny loads ----------
        ident_b = sg.tile([128, 128], BF16)
        make_identity(nc, ident_b)
        ones1 = sg.tile([1, 128], F32)
        nc.vector.memset(ones1, 1.0)
        eps_t = sg.tile([128, 1], F32)
        nc.vector.memset(eps_t, EPS)

        Wz_sb = sg.tile([128, H], F32)
        nc.scalar.dma_start(out=Wz_sb, in_=Wz_d)
        wzv_sb = sg.tile([128, 1], F32)
        nc.scalar.dma_start(out=wzv_sb, in_=wz_d.rearrange("(p o) -> p o", o=1))
        ws_sb = sg.tile([128, CKS], F32)
        nc.scalar.dma_start(out=ws_sb, in_=ws_d.rearrange("(k p) -> p k", p=128))
        bg_sb = sg.tile([1, CS], F32)
        nc.scalar.dma_start(out=bg_sb, in_=bg_d.rearrange("(o c) -> o c", o=1))
        bo_sb = sg.tile([1, CS], F32)
        nc.scalar.dma_start(out=bo_sb, in_=bo_d.rearrange("(o c) -> o c", o=1))

        nc.vector.tensor_scalar_mul(Wz_sb, Wz_sb, wzv_sb)
        Wz_bf = sg.tile([128, H], BF16)
        nc.vector.tensor_copy(out=Wz_bf, in_=Wz_sb)

        # persisted bias tiles: B_01 for jt 0/1 (live before the preamble),
        # B_27 for jt 2..7 (allocated after the preamble pool closes)
        B_01 = sg.tile([128, 2, 128, H], BF16)
        B_tiles = {}

        # ---------- per-jt z pipeline ----------
        def stats_jt(znI, engines):
            msI = msp.tile([128, 128], F32, tag="msI", name="msI")
            for q in range(8):
                sq = sqp.tile([128, SB, CZ], BF16, tag="sq", name="sq")
                src = znI[:, bass.ts(q, SB), :]
                if engines[q] == "s":
                    nc.scalar.square(out=sq, in_=src)
                else:
                    nc.vector.tensor_tensor(out=sq, in0=src, in1=src,
                                            op=OP.mult)
                tA = trp.tile([128, SB, 64], BF16, tag="tA")
                nc.vector.tensor_tensor(
                    out=tA, in0=sq[:, :, 0:64], in1=sq[:, :, 64:128], op=OP.add)
                tB = trp.tile([128, SB, 32], BF16, tag="tB")
                nc.vector.tensor_tensor(
                    out=tB, in0=tA[:, :, 0:32], in1=tA[:, :, 32:64], op=OP.add)
                tC = trp.tile([128, SB, 16], BF16, tag="tC")
                nc.vector.tensor_tensor(
                    out=tC, in0=tB[:, :, 0:16], in1=tB[:, :, 16:32], op=OP.add)
                nc.vector.tensor_reduce(
                    out=msI[:, bass.ts(q, SB)], in_=tC,
                    axis=mybir.AxisListType.X, op=OP.add)
            nc.scalar.activation(out=msI, in_=msI, func=AF.Sqrt,
                                 bias=eps_t, scale=float(1.0 / CZ))
            nc.vector.reciprocal(out=msI, in_=msI)
            return msI

        SPLIT = tuple("s" * 4 + "v" * 4)   # ScalarE 4 chunks, DVE 4
        ALLDVE = tuple("v" * 8)

        def z_step(jt, B_dst, jd, stat_engines):
            znI = zn_tiles.pop(jt)
            zt = ztp.tile([128, 128, 128], BF16, tag="zt", name="zt")
            nc.sync.dma_start(out=zt, in_=znI, transpose=True)
            msI = stats_jt(znI, stat_engines)
            if jt + 2 < JT:
                issue_cast(jt + 2)
            for b in range(RB // BI):
                j0 = b * BI
                b_ps = bpp.tile([128, BI, H], F32, tag="bps", name="b_ps")
                for jj in range(BI):
                    _mm(nc, b_ps[:, jj, :], zt[:, j0 + jj, :], Wz_bf,
                        jj == 0, jj == BI - 1)
                rs_b = bass.AP(
                    tensor=msI.tensor,
                    offset=msI.offset + j0,
                    ap=[msI.ap[0], [1, BI], [0, H]])
                nc.vector.tensor_tensor(
                    out=B_dst[:, jd, j0:j0 + BI, :], in0=b_ps, in1=rs_b,
                    op=OP.mult)

        # prime the z-pipe for jt 0/1 before any preamble work
        z_step(0, B_01, 0, SPLIT)
        z_step(1, B_01, 1, SPLIT)
        B_tiles[0] = (B_01, 0)
        B_tiles[1] = (B_01, 1)

        # ---------- preamble: rmsnorm(s), q/k/v/g (bf16) ----------
        kT = sg.tile([128, CKS, S], BF16)    # [hd_in_chunk, chunk, j]
        v_sb = sg.tile([128, JT, CS], BF16)  # [j_in_tile, jt, hd]
        qT = sg.tile([128, CKS, 128], BF16)  # [hd_in_chunk, chunk, i_loc]
        g_sb = sg.tile([128, CS], F32)

        with tc.tile_pool(name="pre", bufs=1) as pre, \
             tc.tile_pool(name="pre_tmp", bufs=2) as pt, \
             tc.tile_pool(name="pre_ps", bufs=2, space="PSUM") as pp:
            def norm_rows(ap, out_bf):
                nsq = pt.tile([128, CS], BF16, tag="nsq")
                msum = pt.tile([128, 1], F32, tag="nms")
                nc.scalar.activation(out=nsq, in_=ap, func=AF.Square,
                                     scale=float(1.0 / np.sqrt(CS)),
                                     accum_out=msum)
                nc.scalar.activation(out=msum, in_=msum, func=AF.Sqrt,
                                     bias=eps_t, scale=1.0)
                nc.vector.reciprocal(out=msum, in_=msum)
                nc.vector.scalar_tensor_tensor(
                    out=out_bf, in0=ap, scalar=1.0,
                    in1=bass.AP(tensor=msum.tensor, offset=msum.offset,
                                ap=[msum.ap[0], [0, CS]]),
                    op0=OP.mult, op1=OP.mult)

            s_rT = pre.tile([128, CKS, S], BF16)    # [c, k, i]
            s_rTl = pre.tile([128, CKS, 128], BF16)  # [c, k, local i]
            ncopy = 0

            def drain(dst, src):
                nonlocal ncopy
                if ncopy % 2 == 0:
                    nc.vector.tensor_copy(out=dst, in_=src)
                else:
                    nc.scalar.copy(out=dst, in_=src)
                ncopy += 1

            # normalize s row-tile by row-tile, transpose immediately
            for t in range(JT):
                s_tmp = pt.tile([128, CS], F32, tag="s_tmp")
                nc.scalar.dma_start(
                    out=s_tmp,
                    in_=s_full.rearrange("(t p) c -> p t c", p=128)[:, t, :])
                s_rb = pt.tile([128, CS], BF16, tag="s_rb")
                norm_rows(s_tmp, s_rb)
                for k in range(CKS):
                    ps = pp.tile([128, 128], BF16, tag="tp")
                    _mm(nc, ps, s_rb[:, bass.ts(k, 128)], ident_b, True, True,
                        is_transpose=True)
                    drain(s_rT[:, k, bass.ts(t, 128)], ps)
            s_tmp = pt.tile([128, CS], F32, tag="s_tmp")
            nc.scalar.dma_start(out=s_tmp, in_=s_loc)
            s_rlb = pre.tile([128, CS], BF16)
            norm_rows(s_tmp, s_rlb)
            for k in range(CKS):
                ps = pp.tile([128, 128], BF16, tag="tp")
                _mm(nc, ps, s_rlb[:, bass.ts(k, 128)], ident_b, True, True,
                    is_transpose=True)
                drain(s_rTl[:, k, :], ps)

            # fold w_s into s^T (per-partition scalars after transpose)
            for k in range(CKS):
                nc.vector.tensor_scalar_mul(
                    s_rT[:, k, :], s_rT[:, k, :], ws_sb[:, k:k + 1])
                nc.vector.tensor_scalar_mul(
                    s_rTl[:, k, :], s_rTl[:, k, :], ws_sb[:, k:k + 1])

            def load_w(dram):
                wb = pre.tile([128, CKS, CS], BF16, tag="wb", name="wb")
                nc.gpsimd.dma_start(
                    out=wb, in_=dram.rearrange("(k p) c -> p k c", p=128))
                return wb

            wb = load_w(Wq_d)
            for k in range(CKS):
                ps = pp.tile([128, 512], F32, tag="big")
                for ck in range(CKS):
                    _mm(nc, ps[:, 0:128], wb[:, ck, bass.ts(k, 128)],
                        s_rTl[:, ck, :], ck == 0, ck == CKS - 1)
                nc.scalar.mul(out=qT[:, k, :], in_=ps[:, 0:128],
                              mul=float(INVD))
            wb = load_w(Wk_d)
            for k in range(CKS):
                for half in range(2):
                    ps2 = pp.tile([128, 512], F32, tag="big")
                    for ck in range(CKS):
                        _mm(nc, ps2, wb[:, ck, bass.ts(k, 128)],
                            s_rT[:, ck, bass.ts(half, 512)], ck == 0,
                            ck == CKS - 1)
                    drain(kT[:, k, bass.ts(half, 512)], ps2)
            wb = load_w(Wv_d)
            for jc in range(JT):
                ps2 = pp.tile([128, 512], F32, tag="big")
                for ck in range(CKS):
                    _mm(nc, ps2[:, 0:CS], s_rT[:, ck, bass.ts(jc, 128)],
                        wb[:, ck, :], ck == 0, ck == CKS - 1)
                drain(v_sb[:, jc, :], ps2[:, 0:CS])
            wb = load_w(Wg_d)
            ps2 = pp.tile([128, 512], F32, tag="big")
            for ck in range(CKS):
                _mm(nc, ps2[:, 0:CS], s_rTl[:, ck, :], wb[:, ck, :],
                    ck == 0, False)
            _mm(nc, ps2[:, 0:CS], ones1, bg_sb, False, True)
            nc.scalar.copy(out=g_sb, in_=ps2[:, 0:CS])

        # ---------- remaining z tiles ----------
        B_27 = sg.tile([128, JT - 2, 128, H], BF16)
        for jt in range(2, JT):
            z_step(jt, B_27, jt - 2, ALLDVE if jt < 4 else SPLIT)
            B_tiles[jt] = (B_27, jt - 2)
        zctx.close()

        # ---------- scores + attention tail ----------
        HB = 6   # heads per transpose batch
        with tc.tile_pool(name="att_sb", bufs=2) as asb, \
             tc.tile_pool(name="attT_sb", bufs=2) as atsb, \
             tc.tile_pool(name="att16", bufs=2) as a16p, \
             tc.tile_pool(name="den_sb", bufs=2) as dsb, \
             tc.tile_pool(name="tailw", bufs=1) as tlw, \
             tc.tile_pool(name="qk_ps", bufs=2, space="PSUM") as qkp, \
             tc.tile_pool(name="o_ps", bufs=2, space="PSUM") as opp, \
             tc.tile_pool(name="fin_ps", bufs=1, space="PSUM") as fpp:
            mask_bf = tlw.tile([128, S], BF16)
            mi = tlw.tile([128, S], I32)
            nc.scalar.dma_start(out=mi, in_=zm_d)
            nc.vector.tensor_copy(out=mask_bf, in_=mi)
            wo_f = tlw.tile([128, CKS, CS], F32, name="w_Wo")
            nc.scalar.dma_start(
                out=wo_f, in_=Wo_d.rearrange("(k p) c -> p k c", p=128))
            oT_sb = tlw.tile([128, CKS, 128], F32)  # [hd_in_chunk, chunk, i]

            for batch in range(H // HB):
                attb = asb.tile([128, HB, JT, 128], BF16, tag="attb",
                                name="attb")
                for hh in range(HB):
                    h = batch * HB + hh
                    ck, hp = divmod(h, 4)
                    qk = qkp.tile([128, S], F32, tag="qk", name="qk")
                    for half in range(2):
                        _mm(nc, qk[:, bass.ts(half, 512)],
                            qT[bass.ts(hp, 32), ck, :],
                            kT[bass.ts(hp, 32), ck, bass.ts(half, 512)],
                            True, True, tile_position=(32 * hp, 0))
                    att16 = a16p.tile([128, S], F16, tag="a16", name="att16")
                    for jt in range(JT):
                        Bt, jd = B_tiles[jt]
                        b_slice = bass.AP(
                            tensor=Bt.tensor,
                            offset=Bt.offset + (jd * 128) * H + h,
                            ap=[Bt.ap[0], [H, 128]])
                        nc.vector.scalar_tensor_tensor(
                            out=att16[:, bass.ts(jt, 128)],
                            in0=qk[:, bass.ts(jt, 128)], scalar=1.0,
                            in1=b_slice, op0=OP.mult, op1=OP.add)
                    att = attb[:, hh, :, :]
                    nc.scalar.activation(out=att, in_=att16.rearrange(
                        "p (t j) -> p t j", j=128), func=AF.Exp)
                    den = dsb.tile([128, 1], F32, tag="den", name="den")
                    nc.vector.scalar_tensor_tensor(
                        out=att, in0=att, scalar=1.0,
                        in1=mask_bf.rearrange("p (t j) -> p t j", j=128),
                        op0=OP.mult, op1=OP.mult, accum_out=den)
                    nc.vector.reciprocal(out=den, in_=den)
                    nc.vector.tensor_scalar_mul(att, att, den)
                atT = atsb.tile([128, HB, JT, 128], BF16, tag="atT",
                                name="atT")
                nc.sync.dma_start(out=atT, in_=attb, transpose=True)
                for hh in range(HB):
                    h = batch * HB + hh
                    o_ps = opp.tile([32, 128], F32, tag="o", name="o_ps")
                    for jc in range(JT):
                        _mm(nc, o_ps, v_sb[:, jc, bass.ts(h, 32)],
                            atT[:, hh, jc, :], jc == 0, jc == JT - 1)
                    ck, hp = divmod(h, 4)
                    nc.scalar.copy(out=oT_sb[bass.ts(hp, 32), ck, :], in_=o_ps)

            # ---------- output ----------
            fin = fpp.tile([128, CS], F32, tag="fin")
            for k in range(CKS):
                _mm(nc, fin[:, 0:CS], oT_sb[:, k, :], wo_f[:, k, :],
                    k == 0, False)
            _mm(nc, fin[:, 0:CS], ones1, bo_sb, False, True)
            out_sb = tlw.tile([128, CS], F32)
            nc.vector.tensor_tensor(out=out_sb, in0=fin[:, 0:CS], in1=g_sb,
                                    op=OP.mult)
            nc.sync.dma_start(out=out_d, in_=out_sb)

    nc.compile()
    return nc


_NC_CACHE = None


def _get_nc():
    global _NC_CACHE
    if _NC_CACHE is None:
        nc = bacc.Bacc("TRN2", target_bir_lowering=False, debug=False,
                       enable_asserts=False)
        _NC_CACHE = build(nc)
    return _NC_CACHE


def make_in_maps(s, z, z_mask, w_s, w_z, Wz, Wq, Wk, Wv, Wg, bg, Wo, bo):
    f = lambda a: np.ascontiguousarray(np.asarray(a), dtype=np.float32)
    s = f(s)
    shared = dict(s=s, w_s=f(w_s), w_z=f(w_z), Wz=f(Wz), Wq=f(Wq), Wk=f(Wk),
                  Wv=f(Wv), Wg=f(Wg), bg=f(bg), Wo=f(Wo), bo=f(bo))
    zmask = np.ascontiguousarray(np.asarray(z_mask), dtype=np.int32)
    z = f(z)
    in_maps = []
    for c in range(NCORES):
        r0, r1 = c * RB, (c + 1) * RB
        m = dict(shared)
        m["s_loc"] = np.ascontiguousarray(s[r0:r1])
        m["z"] = np.ascontiguousarray(z[r0:r1])
        m["z_mask"] = np.ascontiguousarray(zmask[r0:r1])
        in_maps.append(m)
    return in_maps


def kernel(**inputs):
    from concourse import bass_utils
    nc = _get_nc()
    in_maps = make_in_maps(**inputs)
    res = bass_utils.run_bass_kernel_spmd(nc, in_maps, core_ids=list(range(NCORES)))
    out = np.concatenate([res.results[c]["out"] for c in range(NCORES)], axis=0)
    return out.astype(np.float32)


# revision 26
# speedup vs baseline: 1.1919x; 1.0373x over previous
"""AttentionWithPairBias distributed Trainium2 kernel (8 NeuronCores).

Sequence-parallel: core c owns query rows i in [128c, 128(c+1)).
Per core: z shard [128, 1024, 128] f32 (64MB -> memory roofline), s and
weights replicated. No collectives.

v4 structure (trace-driven; baseline 507us):
- z-pipe primed before the preamble: transposes + stats + bias matmuls
  for jt 0/1 are emitted first so no engine FIFO puts preamble work in
  front of them (v3 lost ~120us to bias(0) queued behind the preamble).
- Scores (qk) moved out of the z-loop into the tail as N=512 matmuls
  against the persisted per-jt bias tiles B_all; z-loop PE work is pure
  bias matmuls. qk stays f32 in PSUM until the exp.
- z^2 stats: squares ScalarE/DVE split + DVE bf16 halving tree (2x)
  instead of 1x tensor_reduce; stats for jt 2/3 are all-DVE so they
  don't queue behind preamble ScalarE work.
- Preamble: s normalized in row-tile chunks (f32 temp -> bf16), w_s
  folded into s^T after the PE transpose, weights cast f32->bf16 by the
  SWDGE DMA itself, all preamble DMAs on the scalar HWDGE ring; sync
  ring carries only the 8 z transposes (casts are SWDGE).
- Tail: per-head exp/mask/normalize, 2 batched 6-head xbar transposes,
  attn @ v, then (o @ Wo + bo) * g.
"""

import os
from contextlib import ExitStack

import numpy as np

import concourse.bass as bass
import concourse.bacc as bacc
import concourse.tile as tile
import concourse.mybir as mybir
from concourse.masks import make_identity

S = 1024
CS = 384
CZ = 128
D = 32
H = 12
NCORES = 8
RB = S // NCORES  # 128 query rows per core
JT = S // 128     # 8 column tiles
CKS = CS // 128   # 3 contraction chunks of s-dim
EPS = 1e-5
INVD = 1.0 / np.sqrt(D)

F32 = mybir.dt.float32
BF16 = mybir.dt.bfloat16
F16 = mybir.dt.float16
I32 = mybir.dt.int32
AF = mybir.ActivationFunctionType
OP = mybir.AluOpType

BI = 32   # bias psum batch (32*12*4B = 1.5KB <= bank)
SB = 16   # stats chunk (j per square/tree pass)


def _mm(nc, out, lhsT, rhs, start, stop, **kw):
    nc.tensor.matmul(out, lhsT, rhs, start=start, stop=stop, **kw)


def build(nc):
    s_full = nc.dram_tensor("s", [S, CS], F32, kind="ExternalInput").ap()
    s_loc = nc.dram_tensor("s_loc", [RB, CS], F32, kind="ExternalInput").ap()
    z_d = nc.dram_tensor("z", [RB, S, CZ], F32, kind="ExternalInput").ap()
    zm_d = nc.dram_tensor("z_mask", [RB, S], I32, kind="ExternalInput").ap()
    ws_d = nc.dram_tensor("w_s", [CS], F32, kind="ExternalInput").ap()
    wz_d = nc.dram_tensor("w_z", [CZ], F32, kind="ExternalInput").ap()
    Wz_d = nc.dram_tensor("Wz", [CZ, H], F32, kind="ExternalInput").ap()
    Wq_d = nc.dram_tensor("Wq", [CS, CS], F32, kind="ExternalInput").ap()
    Wk_d = nc.dram_tensor("Wk", [CS, CS], F32, kind="ExternalInput").ap()
    Wv_d = nc.dram_tensor("Wv", [CS, CS], F32, kind="ExternalInput").ap()
    Wg_d = nc.dram_tensor("Wg", [CS, CS], F32, kind="ExternalInput").ap()
    bg_d = nc.dram_tensor("bg", [CS], F32, kind="ExternalInput").ap()
    Wo_d = nc.dram_tensor("Wo", [CS, CS], F32, kind="ExternalInput").ap()
    bo_d = nc.dram_tensor("bo", [CS], F32, kind="ExternalInput").ap()
    out_d = nc.dram_tensor("out", [RB, CS], F32, kind="ExternalOutput").ap()

    with tile.TileContext(nc) as tc, ExitStack() as ctx:
        sg = ctx.enter_context(tc.tile_pool(name="singles", bufs=1))
        zctx = ExitStack()
        znp = zctx.enter_context(tc.tile_pool(name="znat", bufs=2))
        ztp = zctx.enter_context(tc.tile_pool(name="znT", bufs=2))
        sqp = zctx.enter_context(tc.tile_pool(name="sqp", bufs=2))
        trp = zctx.enter_context(tc.tile_pool(name="trp", bufs=1))
        msp = zctx.enter_context(tc.tile_pool(name="msp", bufs=2))
        bpp = zctx.enter_context(tc.tile_pool(name="bias_ps", bufs=4,
                                              space="PSUM"))

        zn_tiles = {}

        def issue_cast(jt):
            znI = znp.tile([128, 128, CZ], BF16, tag="zn", name="znI")
            nc.gpsimd.dma_start(out=znI, in_=z_d[:, bass.ts(jt, 128), :])
            zn_tiles[jt] = znI

        issue_cast(0)
        issue_cast(1)

        # ---------- constants / tiny loads ----------
        ident_b = sg.tile([128, 128], BF16)
        make_identity(nc, ident_b)
        ones1 = sg.tile([1, 128], F32)
        nc.vector.memset(ones1, 1.0)
        eps_t = sg.tile([128, 1], F32)
        nc.vector.memset(eps_t, EPS)

        Wz_sb = sg.tile([128, H], F32)
        nc.scalar.dma_start(out=Wz_sb, in_=Wz_d)
        wzv_sb = sg.tile([128, 1], F32)
        nc.scalar.dma_start(out=wzv_sb, in_=wz_d.rearrange("(p o) -> p o", o=1))
        ws_sb = sg.tile([128, CKS], F32)
        nc.scalar.dma_start(out=ws_sb, in_=ws_d.rearrange("(k p) -> p k", p=128))
        bg_sb = sg.tile([1, CS], F32)
        nc.scalar.dma_start(out=bg_sb, in_=bg_d.rearrange("(o c) -> o c", o=1))
        bo_sb = sg.tile([1, CS], F32)
        nc.scalar.dma_start(out=bo_sb, in_=bo_d.rearrange("(o c) -> o c", o=1))

        nc.vector.tensor_scalar_mul(Wz_sb, Wz_sb, wzv_sb)
        Wz_bf = sg.tile([128, H], BF16)
        nc.vector.tensor_copy(out=Wz_bf, in_=Wz_sb)

        # persisted bias tiles: B_01 for jt 0/1 (live before the preamble),
        # B_27 for jt 2..7 (allocated after the preamble pool closes)
        B_01 = sg.tile([128, 2, 128, H], BF16)
        B_tiles = {}

        # ---------- per-jt z pipeline ----------
        def stats_jt(znI, engines):
            msI = msp.tile([128, 128], F32, tag="msI", name="msI")
            for q in range(8):
                sq = sqp.tile([128, SB, CZ], BF16, tag="sq", name="sq")
                src = znI[:, bass.ts(q, SB), :]
                if engines[q] == "s":
                    nc.scalar.square(out=sq, in_=src)
                else:
                    nc.vector.tensor_tensor(out=sq, in0=src, in1=src,
                                            op=OP.mult)
                tA = trp.tile([128, SB, 64], BF16, tag="tA")
                nc.vector.tensor_tensor(
                    out=tA, in0=sq[:, :, 0:64], in1=sq[:, :, 64:128], op=OP.add)
                tB = trp.tile([128, SB, 32], BF16, tag="tB")
                nc.vector.tensor_tensor(
                    out=tB, in0=tA[:, :, 0:32], in1=tA[:, :, 32:64], op=OP.add)
                tC = trp.tile([128, SB, 16], BF16, tag="tC")
                nc.vector.tensor_tensor(
                    out=tC, in0=tB[:, :, 0:16], in1=tB[:, :, 16:32], op=OP.add)
                nc.vector.tensor_reduce(
                    out=msI[:, bass.ts(q, SB)], in_=tC,
                    axis=mybir.AxisListType.X, op=OP.add)
            nc.scalar.activation(out=msI, in_=msI, func=AF.Sqrt,
                                 bias=eps_t, scale=float(1.0 / CZ))
            nc.vector.reciprocal(out=msI, in_=msI)
            return msI

        SPLIT = tuple("s" * 4 + "v" * 4)   # ScalarE 4 chunks, DVE 4
        ALLDVE = tuple("v" * 8)
        ALLSC = tuple("s" * 8)

        def z_step(jt, B_dst, jd, stat_engines):
            znI = zn_tiles.pop(jt)
            zt = ztp.tile([128, 128, 128], BF16, tag="zt", name="zt")
            nc.sync.dma_start(out=zt, in_=znI, transpose=True)
            msI = stats_jt(znI, stat_engines)
            if jt + 2 < JT:
                issue_cast(jt + 2)
            for b in range(RB // BI):
                j0 = b * BI
                b_ps = bpp.tile([128, BI, H], F32, tag="bps", name="b_ps")
                for jj in range(BI):
                    _mm(nc, b_ps[:, jj, :], zt[:, j0 + jj, :], Wz_bf,
                        jj == 0, jj == BI - 1)
                rs_b = bass.AP(
                    tensor=msI.tensor,
                    offset=msI.offset + j0,
                    ap=[msI.ap[0], [1, BI], [0, H]])
                nc.vector.tensor_tensor(
                    out=B_dst[:, jd, j0:j0 + BI, :], in0=b_ps, in1=rs_b,
                    op=OP.mult)

        # prime the z-pipe for jt 0/1 before any preamble work
        z_step(0, B_01, 0, SPLIT)
        z_step(1, B_01, 1, SPLIT)
        B_tiles[0] = (B_01, 0)
        B_tiles[1] = (B_01, 1)

        # ---------- preamble: rmsnorm(s), q/k/v/g (bf16) ----------
        kT = sg.tile([128, CKS, S], BF16)    # [hd_in_chunk, chunk, j]
        v_sb = sg.tile([128, JT, CS], BF16)  # [j_in_tile, jt, hd]
        qT = sg.tile([128, CKS, 128], BF16)  # [hd_in_chunk, chunk, i_loc]
        g_sb = sg.tile([128, CS], F32)

        with tc.tile_pool(name="pre", bufs=1) as pre, \
             tc.tile_pool(name="pre_tmp", bufs=2) as pt, \
             tc.tile_pool(name="pre_ps", bufs=2, space="PSUM") as pp:
            def norm_rows(ap, out_bf):
                nsq = pt.tile([128, CS], BF16, tag="nsq")
                msum = pt.tile([128, 1], F32, tag="nms")
                nc.scalar.activation(out=nsq, in_=ap, func=AF.Square,
                                     scale=float(1.0 / np.sqrt(CS)),
                                     accum_out=msum)
                nc.scalar.activation(out=msum, in_=msum, func=AF.Sqrt,
                                     bias=eps_t, scale=1.0)
                nc.vector.reciprocal(out=msum, in_=msum)
                nc.vector.scalar_tensor_tensor(
                    out=out_bf, in0=ap, scalar=1.0,
                    in1=bass.AP(tensor=msum.tensor, offset=msum.offset,
                                ap=[msum.ap[0], [0, CS]]),
                    op0=OP.mult, op1=OP.mult)

            s_rT = pre.tile([128, CKS, S], BF16)    # [c, k, i]
            s_rTl = pre.tile([128, CKS, 128], BF16)  # [c, k, local i]
            ncopy = 0

            def drain(dst, src):
                nonlocal ncopy
                if ncopy % 2 == 0:
                    nc.vector.tensor_copy(out=dst, in_=src)
                else:
                    nc.scalar.copy(out=dst, in_=src)
                ncopy += 1

            # normalize s row-tile by row-tile, transpose immediately
            for t in range(JT):
                s_tmp = pt.tile([128, CS], F32, tag="s_tmp")
                nc.scalar.dma_start(
                    out=s_tmp,
                    in_=s_full.rearrange("(t p) c -> p t c", p=128)[:, t, :])
                s_rb = pt.tile([128, CS], BF16, tag="s_rb")
                norm_rows(s_tmp, s_rb)
                for k in range(CKS):
                    ps = pp.tile([128, 128], BF16, tag="tp")
                    _mm(nc, ps, s_rb[:, bass.ts(k, 128)], ident_b, True, True,
                        is_transpose=True)
                    drain(s_rT[:, k, bass.ts(t, 128)], ps)
            s_tmp = pt.tile([128, CS], F32, tag="s_tmp")
            nc.scalar.dma_start(out=s_tmp, in_=s_loc)
            s_rlb = pre.tile([128, CS], BF16)
            norm_rows(s_tmp, s_rlb)
            for k in range(CKS):
                ps = pp.tile([128, 128], BF16, tag="tp")
                _mm(nc, ps, s_rlb[:, bass.ts(k, 128)], ident_b, True, True,
                    is_transpose=True)
                drain(s_rTl[:, k, :], ps)

            # fold w_s into s^T (per-partition scalars after transpose)
            for k in range(CKS):
                nc.vector.tensor_scalar_mul(
                    s_rT[:, k, :], s_rT[:, k, :], ws_sb[:, k:k + 1])
                nc.vector.tensor_scalar_mul(
                    s_rTl[:, k, :], s_rTl[:, k, :], ws_sb[:, k:k + 1])

            def load_w(dram):
                wb = pre.tile([128, CKS, CS], BF16, tag="wb", name="wb")
                nc.gpsimd.dma_start(
                    out=wb, in_=dram.rearrange("(k p) c -> p k c", p=128))
                return wb

            wb = load_w(Wq_d)
            for k in range(CKS):
                ps = pp.tile([128, 512], F32, tag="big")
                for ck in range(CKS):
                    _mm(nc, ps[:, 0:128], wb[:, ck, bass.ts(k, 128)],
                        s_rTl[:, ck, :], ck == 0, ck == CKS - 1)
                nc.scalar.mul(out=qT[:, k, :], in_=ps[:, 0:128],
                              mul=float(INVD))
            wb = load_w(Wk_d)
            for k in range(CKS):
                for half in range(2):
                    ps2 = pp.tile([128, 512], F32, tag="big")
                    for ck in range(CKS):
                        _mm(nc, ps2, wb[:, ck, bass.ts(k, 128)],
                            s_rT[:, ck, bass.ts(half, 512)], ck == 0,
                            ck == CKS - 1)
                    drain(kT[:, k, bass.ts(half, 512)], ps2)
            wb = load_w(Wv_d)
            for jc in range(JT):
                ps2 = pp.tile([128, 512], F32, tag="big")
                for ck in range(CKS):
                    _mm(nc, ps2[:, 0:CS], s_rT[:, ck, bass.ts(jc, 128)],
                        wb[:, ck, :], ck == 0, ck == CKS - 1)
                drain(v_sb[:, jc, :], ps2[:, 0:CS])
            wb = load_w(Wg_d)
            ps2 = pp.tile([128, 512], F32, tag="big")
            for ck in range(CKS):
                _mm(nc, ps2[:, 0:CS], s_rTl[:, ck, :], wb[:, ck, :],
                    ck == 0, False)
            _mm(nc, ps2[:, 0:CS], ones1, bg_sb, False, True)
            nc.scalar.copy(out=g_sb, in_=ps2[:, 0:CS])

        # ---------- remaining z tiles ----------
        B_27 = sg.tile([128, JT - 2, 128, H], BF16)
        for jt in range(2, JT):
            z_step(jt, B_27, jt - 2, ALLDVE if jt < 4 else ALLSC)
            B_tiles[jt] = (B_27, jt - 2)
        zctx.close()

        # ---------- scores + attention tail ----------
        HB = 6   # heads per transpose batch
        with tc.tile_pool(name="att_sb", bufs=2) as asb, \
             tc.tile_pool(name="attT_sb", bufs=2) as atsb, \
             tc.tile_pool(name="att16", bufs=2) as a16p, \
             tc.tile_pool(name="den_sb", bufs=2) as dsb, \
             tc.tile_pool(name="tailw", bufs=1) as tlw, \
             tc.tile_pool(name="qk_ps", bufs=2, space="PSUM") as qkp, \
             tc.tile_pool(name="o_ps", bufs=2, space="PSUM") as opp, \
             tc.tile_pool(name="fin_ps", bufs=1, space="PSUM") as fpp:
            mask_bf = tlw.tile([128, S], BF16)
            mi = tlw.tile([128, S], I32)
            nc.scalar.dma_start(out=mi, in_=zm_d)
            nc.vector.tensor_copy(out=mask_bf, in_=mi)
            wo_f = tlw.tile([128, CKS, CS], F32, name="w_Wo")
            nc.scalar.dma_start(
                out=wo_f, in_=Wo_d.rearrange("(k p) c -> p k c", p=128))
            oT_sb = tlw.tile([128, CKS, 128], F32)  # [hd_in_chunk, chunk, i]

            for batch in range(H // HB):
                attb = asb.tile([128, HB, JT, 128], BF16, tag="attb",
                                name="attb")
                for hh in range(HB):
                    h = batch * HB + hh
                    ck, hp = divmod(h, 4)
                    qk = qkp.tile([128, S], F32, tag="qk", name="qk")
                    for half in range(2):
                        _mm(nc, qk[:, bass.ts(half, 512)],
                            qT[bass.ts(hp, 32), ck, :],
                            kT[bass.ts(hp, 32), ck, bass.ts(half, 512)],
                            True, True, tile_position=(32 * hp, 0))
                    att16 = a16p.tile([128, S], F16, tag="a16", name="att16")
                    for Bt, njt, j0 in ((B_01, 2, 0), (B_27, JT - 2, 256)):
                        b_slice = bass.AP(
                            tensor=Bt.tensor,
                            offset=Bt.offset + h,
                            ap=[Bt.ap[0], [128 * H, njt], [H, 128]])
                        qk_s = bass.AP(
                            tensor=qk.tensor, offset=qk.offset + j0,
                            ap=[qk.ap[0], [128, njt], [1, 128]])
                        a16_s = bass.AP(
                            tensor=att16.tensor, offset=att16.offset + j0,
                            ap=[att16.ap[0], [128, njt], [1, 128]])
                        nc.vector.scalar_tensor_tensor(
                            out=a16_s, in0=qk_s, scalar=1.0,
                            in1=b_slice, op0=OP.mult, op1=OP.add)
                    att = attb[:, hh, :, :]
                    nc.scalar.activation(out=att, in_=att16.rearrange(
                        "p (t j) -> p t j", j=128), func=AF.Exp)
                    den = dsb.tile([128, 1], F32, tag="den", name="den")
                    nc.vector.scalar_tensor_tensor(
                        out=att, in0=att, scalar=1.0,
                        in1=mask_bf.rearrange("p (t j) -> p t j", j=128),
                        op0=OP.mult, op1=OP.mult, accum_out=den)
                    nc.vector.reciprocal(out=den, in_=den)
                    nc.scalar.mul(out=att, in_=att, mul=den)
                atT = atsb.tile([128, HB, JT, 128], BF16, tag="atT",
                                name="atT")
                nc.sync.dma_start(out=atT, in_=attb, transpose=True)
                for hh in range(HB):
                    h = batch * HB + hh
                    o_ps = opp.tile([32, 128], F32, tag="o", name="o_ps")
                    for jc in range(JT):
                        _mm(nc, o_ps, v_sb[:, jc, bass.ts(h, 32)],
                            atT[:, hh, jc, :], jc == 0, jc == JT - 1)
                    ck, hp = divmod(h, 4)
                    nc.scalar.copy(out=oT_sb[bass.ts(hp, 32), ck, :], in_=o_ps)

            # ---------- output ----------
            fin = fpp.tile([128, CS], F32, tag="fin")
            for k in range(CKS):
                _mm(nc, fin[:, 0:CS], oT_sb[:, k, :], wo_f[:, k, :],
                    k == 0, False)
            _mm(nc, fin[:, 0:CS], ones1, bo_sb, False, True)
            out_sb = tlw.tile([128, CS], F32)
            nc.vector.tensor_tensor(out=out_sb, in0=fin[:, 0:CS], in1=g_sb,
                                    op=OP.mult)
            nc.sync.dma_start(out=out_d, in_=out_sb)

    nc.compile()
    return nc


_NC_CACHE = None


def _get_nc():
    global _NC_CACHE
    if _NC_CACHE is None:
        nc = bacc.Bacc("TRN2", target_bir_lowering=False, debug=False,
                       enable_asserts=False)
        _NC_CACHE = build(nc)
    return _NC_CACHE


def make_in_maps(s, z, z_mask, w_s, w_z, Wz, Wq, Wk, Wv, Wg, bg, Wo, bo):
    f = lambda a: np.ascontiguousarray(np.asarray(a), dtype=np.float32)
    s = f(s)
    shared = dict(s=s, w_s=f(w_s), w_z=f(w_z), Wz=f(Wz), Wq=f(Wq), Wk=f(Wk),
                  Wv=f(Wv), Wg=f(Wg), bg=f(bg), Wo=f(Wo), bo=f(bo))
    zmask = np.ascontiguousarray(np.asarray(z_mask), dtype=np.int32)
    z = f(z)
    in_maps = []
    for c in range(NCORES):
        r0, r1 = c * RB, (c + 1) * RB
        m = dict(shared)
        m["s_loc"] = np.ascontiguousarray(s[r0:r1])
        m["z"] = np.ascontiguousarray(z[r0:r1])
        m["z_mask"] = np.ascontiguousarray(zmask[r0:r1])
        in_maps.append(m)
    return in_maps


def kernel(**inputs):
    from concourse import bass_utils
    nc = _get_nc()
    in_maps = make_in_maps(**inputs)
    res = bass_utils.run_bass_kernel_spmd(nc, in_maps, core_ids=list(range(NCORES)))
    out = np.concatenate([res.results[c]["out"] for c in range(NCORES)], axis=0)
    return out.astype(np.float32)
